# revision 21
# baseline (speedup 1.0000x reference)
"""Trainium2 Bass kernel for nn_BaseLocalInference (co-attention block).

reference:
    energy = a_hat @ b_hat.T                       # [La, Lb]
    wave_a = softmax(energy, dim=1) @ b_hat        # [La, D]
    wave_b = softmax(energy, dim=0).T @ a_hat      # [Lb, D]
    m_a = concat(a_hat, wave_a, a_hat-wave_a, a_hat*wave_a)   # [4*La, D]
    m_b = concat(b_hat, wave_b, b_hat-wave_b, b_hat*wave_b)   # [4*Lb, D]

Sharding (8 cores): core i owns a-rows [512i, 512i+512) and b-rows likewise.
Both softmaxes are computed exactly with no mid-kernel collectives by giving
each core the full "other" matrix:

  phase A (per core): Ea^T = B @ A_i^T            [4096(n) x 512(m)]
      rm[m] = max over n  (partition reduce)      -> exact dim-1 stats
      X = exp(Ea^T - rm)                          (lhsT of wave_a)
      wave_a_i = X.T @ [B | 1] -> [512, 1024(+rowsum)] -> normalize
  phase B: identical with roles of A and B swapped -> wave_b_i.

Precision: the PE's float32r path rounds operands to 11 mantissa bits
(measured), which is too coarse for softmax logits. The energy matmul is
instead computed as a 3-pass split-bf16 product
    E = Ah@Bh + Al@Bh + Ah@Bl      (A = Ah + Al exactly, bf16 parts)
giving ~16-bit operand precision at full PE rate (bf16 = 1 cycle/row).
The host pre-transposes and pre-splits the operands (pure layout work), so
the device runs only matmuls + softmax + elementwise. Wave matmuls run
single-pass float32r (X in [0,1]; ~1e-4 relative).
"""
import os
import sys

sys.path.insert(0, os.path.dirname(os.path.abspath(__file__)))

import numpy as np

import concourse.bass as bass
import concourse.tile as tile
from concourse import mybir
from concourse.bass_utils import run_bass_kernel_spmd

_uid = [0]


def split_multi_waits(nc):
    """This walrus build encodes at most ONE sync wait per instruction
    ("Too many sync wait commands", CoreV3GenImpl setupSyncWait), while Tile's
    scheduler freely attaches several. Hoist all-but-one wait of each
    multi-wait instruction onto same-engine NOPs placed immediately before it
    (engines execute their instructions in block order, so semantics are
    identical)."""
    for fn in nc.m.functions:
        for bb in fn.blocks:
            insts = list(bb.instructions)
            out = []
            changed = False
            for ins in insts:
                si = getattr(ins, "sync_info", None)
                if si is not None and si.on_wait and len(si.on_wait) > 1:
                    changed = True
                    extra = list(si.on_wait[:-1])
                    keep = [si.on_wait[-1]]
                    for w in extra:
                        _uid[0] += 1
                        nop = mybir.InstNoOp(
                            name=f"I-waitsplit-{_uid[0]}",
                            sync_info=mybir.SyncInfo(on_wait=[w], on_update=[]),
                            bass_nofuse=True,
                            engine=ins.engine,
                        )
                        out.append(nop)
                        nc.register_instruction(nop, overwrite=True)
                    si.on_wait.clear()
                    si.on_wait.extend(keep)
                out.append(ins)
            if changed:
                bb.instructions.clear()
                bb.instructions.extend(out)

P = 128          # partitions
S = 512          # slab rows per core
L = 4096         # La = Lb
D = 1024         # feature dim
NB = 8           # cores
FD = 512         # matmul free dim
F32 = mybir.dt.float32
F32R = mybir.dt.float32r
BF16 = mybir.dt.bfloat16


def _emit_half(nc, tc, lhs_h_dram, lhs_l_dram, locTs_h, locTs_l, nat_dram,
               own_slab_dram, out_dram, ones, ones_row, ident_f, tag):
    """One co-attention half. Writes out_dram [3, S, D] = (wave, own-wave, own*wave).

    lhs_h/l_dram: other matrix transposed+split  [D(1024), L(4096)] bf16
    locT_h/l:     own slab transposed+split SBUF [P, 8, S] bf16
    nat_dram:     other matrix natural [L, D] f32r (wave rhs)
    own_slab_dram: own slab natural [S, D] f32r (for diff/prod)
    """
    from contextlib import ExitStack

    with ExitStack() as ctx:
        epool = ctx.enter_context(tc.tile_pool(name=f"E{tag}", bufs=1))
        stats = ctx.enter_context(tc.tile_pool(name=f"stats{tag}", bufs=1))

        # E: fp32 logits (must NOT pass through an f32r store -- f32r writes
        # round to 11 mantissa bits, which would quantize the softmax logits).
        # X = exp(E - rm): values in (0,1], safe to store as f32r for the PE.
        E = epool.tile([P, 32, FD], F32, name=f"Egt{tag}")
        X = epool.tile([P, 32, FD], F32R, name=f"Xgt{tag}")

        # ---- energy: 3-pass split-bf16, E^T tiles [n(128), m(512)] ----
        with ExitStack() as ectx:
            loc = ectx.enter_context(tc.tile_pool(name=f"loc{tag}", bufs=1))
            # own slab transposed+split (energy rhs), loaded per phase
            locT_h = loc.tile([P, 8, S], BF16, name=f"loch{tag}")
            locT_l = loc.tile([P, 8, S], BF16, name=f"locl{tag}")
            nc.sync.dma_start(locT_h[:], locTs_h.rearrange("(c p) m -> p c m", p=P))
            nc.scalar.dma_start(locT_l[:], locTs_l.rearrange("(c p) m -> p c m", p=P))
            lhs_pool = ectx.enter_context(tc.tile_pool(name=f"lhs{tag}", bufs=2))
            eps = ectx.enter_context(tc.tile_pool(name=f"eps{tag}", bufs=4, space="PSUM"))
            for j in range(NB):
                blk_h = lhs_pool.tile([P, 8, FD], BF16, name=f"blkh{tag}", tag="blkh")
                blk_l = lhs_pool.tile([P, 8, FD], BF16, name=f"blkl{tag}", tag="blkl")
                nc.sync.dma_start(
                    blk_h[:],
                    lhs_h_dram[:, j * FD:(j + 1) * FD].rearrange("(c p) n -> p c n", p=P),
                )
                nc.scalar.dma_start(
                    blk_l[:],
                    lhs_l_dram[:, j * FD:(j + 1) * FD].rearrange("(c p) n -> p c n", p=P),
                )
                for jj in range(4):
                    ps = eps.tile([P, FD], F32, name=f"eps{tag}", tag="eps")
                    nsl = slice(jj * P, (jj + 1) * P)
                    for c in range(8):
                        nc.tensor.matmul(ps[:], blk_h[:, c, nsl], locT_h[:, c, :],
                                         start=(c == 0), stop=False)
                        nc.tensor.matmul(ps[:], blk_h[:, c, nsl], locT_l[:, c, :],
                                         start=False, stop=False)
                    for c in range(8):
                        nc.tensor.matmul(ps[:], blk_l[:, c, nsl], locT_h[:, c, :],
                                         start=False, stop=(c == 7))
                    nc.scalar.copy(E[:, j * 4 + jj, :], ps[:])

        # ---- stats: rm[m] = max over n (32 tiles then 128 partitions) ----
        sc = stats.tile([P, 2, FD], F32, name=f"sc{tag}")
        for g in range(2):
            nc.vector.tensor_max(sc[:, g], E[:, 16 * g], E[:, 16 * g + 1])
            for u in range(2, 16):
                nc.vector.tensor_max(sc[:, g], sc[:, g], E[:, 16 * g + u])
        nc.vector.tensor_max(sc[:, 0], sc[:, 0], sc[:, 1])
        # partition reduce via PE transpose + free-dim reduce, then broadcast
        # back across partitions with a K=1 ones-matmul.
        rmrow = stats.tile([1, FD], F32, name=f"rmrow{tag}")
        bc = stats.tile([P, FD], F32, name=f"bc{tag}")
        with tc.tile_pool(name=f"stps{tag}", bufs=2, space="PSUM") as stps:
            for j in range(4):
                tp = stps.tile([P, P], F32, name=f"sttp{tag}", tag="st_tp")
                nc.tensor.transpose(tp[:], sc[:, 0, j * P:(j + 1) * P], ident_f[:])
                rmj = stats.tile([P, 1], F32, name=f"rmj{tag}", tag="rmj", bufs=2)
                nc.vector.reduce_max(rmj[:], tp[:], axis=mybir.AxisListType.X)
                tp2 = stps.tile([1, P], F32, name=f"sttp2{tag}", tag="st_tp2")
                nc.tensor.transpose(tp2[:], rmj[:], ident_f[:])
                nc.scalar.copy(rmrow[0:1, j * P:(j + 1) * P], tp2[:])
            bcps = stps.tile([P, FD], F32, name=f"bcps{tag}", tag="bcps")
            nc.tensor.matmul(bcps[:], ones_row[:], rmrow[:],
                             start=True, stop=True)
            nc.scalar.copy(bc[:], bcps[:])

        # ---- X = exp(E - bc) ----
        for k in range(32):
            nc.vector.tensor_sub(E[:, k], E[:, k], bc[:])
            nc.scalar.activation(
                X[:, k], E[:, k], mybir.ActivationFunctionType.Exp
            )

        # ---- rowsum rs[m] = sum over n of X: DVE tree-add + PE transpose ----
        ssum = sc
        for g in range(2):
            nc.vector.tensor_add(ssum[:, g], X[:, 16 * g].bitcast(F32),
                                 X[:, 16 * g + 1].bitcast(F32))
            for u in range(2, 16):
                nc.vector.tensor_add(ssum[:, g], ssum[:, g],
                                     X[:, 16 * g + u].bitcast(F32))
        nc.vector.tensor_add(ssum[:, 0], ssum[:, 0], ssum[:, 1])

        # ---- wave = X.T @ [nat | 1], rowsum in the extra column ----
        wpool = ctx.enter_context(tc.tile_pool(name=f"w{tag}", bufs=1))
        rhs_pool = ctx.enter_context(tc.tile_pool(name=f"rhs{tag}", bufs=2))
        wave = wpool.tile([P, 4, D], F32, name=f"wave{tag}")
        rsr = wpool.tile([P, 4], F32, name=f"rsr{tag}")
        with tc.tile_pool(name=f"rsps{tag}", bufs=2, space="PSUM") as rsps:
            for mt in range(4):
                rtp = rsps.tile([P, P], F32, name=f"rtp{tag}", tag="rtp")
                nc.tensor.transpose(rtp[:], ssum[:, 0, mt * P:(mt + 1) * P],
                                    ident_f[:])
                rs = wpool.tile([P, 1], F32, name=f"rs{tag}{mt}", tag="rs", bufs=4)
                nc.vector.reduce_sum(rs[:], rtp[:], axis=mybir.AxisListType.X)
                nc.vector.reciprocal(rsr[:, mt:mt + 1], rs[:])
        wps = ctx.enter_context(tc.tile_pool(name=f"wps{tag}", bufs=2, space="PSUM"))
        for dp in range(2):
            psw = [wps.tile([P, FD], F32, name=f"wps{tag}{dp}_{mt}", tag=f"wps{mt}")
                   for mt in range(4)]
            for k4 in range(8):
                rhs = rhs_pool.tile([P, 4, FD], F32R, name=f"rhs{tag}", tag="rhs")
                nc.gpsimd.dma_start(
                    rhs[:],
                    nat_dram[k4 * 4 * P:(k4 + 1) * 4 * P,
                             dp * FD:(dp + 1) * FD].rearrange(
                        "(kk p) f -> p kk f", p=P),
                )
                for kk in range(4):
                    k = k4 * 4 + kk
                    for mt in range(4):
                        nc.tensor.matmul(
                            psw[mt][:], X[:, k, mt * P:(mt + 1) * P], rhs[:, kk],
                            start=(k == 0), stop=(k == 31)
                        )
            for mt in range(4):
                nc.vector.tensor_scalar_mul(
                    wave[:, mt, dp * FD:(dp + 1) * FD], psw[mt][:], rsr[:, mt:mt + 1]
                )

        # ---- outputs: wave, own - wave, own * wave ----
        nc.scalar.dma_start(
            out_dram[0].rearrange("(t p) d -> p t d", p=P), wave[:]
        )
        opool = ctx.enter_context(tc.tile_pool(name=f"o{tag}", bufs=1))
        own_nat = opool.tile([P, 4, D], F32R, name=f"own{tag}", tag="own")
        nc.sync.dma_start(own_nat[:], own_slab_dram.rearrange("(t p) d -> p t d", p=P))
        for mt in range(4):
            dtile = opool.tile([P, D], F32, name=f"d{tag}", tag="dif")
            nc.vector.tensor_sub(dtile[:], own_nat[:, mt].bitcast(F32), wave[:, mt])
            nc.scalar.dma_start(out_dram[1, mt * P:(mt + 1) * P, :], dtile[:])
            ptile = opool.tile([P, D], F32, name=f"p{tag}", tag="prd")
            nc.vector.tensor_mul(ptile[:], own_nat[:, mt].bitcast(F32), wave[:, mt])
            nc.scalar.dma_start(out_dram[2, mt * P:(mt + 1) * P, :], ptile[:])


def build_program():
    from contextlib import ExitStack

    nc = bass.Bass()
    a_full = nc.dram_tensor("a_full", [L, D], F32R, kind="ExternalInput")
    b_full = nc.dram_tensor("b_full", [L, D], F32R, kind="ExternalInput")
    a_slab = nc.dram_tensor("a_slab", [S, D], F32R, kind="ExternalInput")
    b_slab = nc.dram_tensor("b_slab", [S, D], F32R, kind="ExternalInput")
    at_h = nc.dram_tensor("at_h", [D, L], BF16, kind="ExternalInput")
    at_l = nc.dram_tensor("at_l", [D, L], BF16, kind="ExternalInput")
    bt_h = nc.dram_tensor("bt_h", [D, L], BF16, kind="ExternalInput")
    bt_l = nc.dram_tensor("bt_l", [D, L], BF16, kind="ExternalInput")
    ats_h = nc.dram_tensor("ats_h", [D, S], BF16, kind="ExternalInput")
    ats_l = nc.dram_tensor("ats_l", [D, S], BF16, kind="ExternalInput")
    bts_h = nc.dram_tensor("bts_h", [D, S], BF16, kind="ExternalInput")
    bts_l = nc.dram_tensor("bts_l", [D, S], BF16, kind="ExternalInput")
    ident_in = nc.dram_tensor("ident", [P, P], F32, kind="ExternalInput")
    ma = nc.dram_tensor("ma", [3, S, D], F32, kind="ExternalOutput")
    mb = nc.dram_tensor("mb", [3, S, D], F32, kind="ExternalOutput")

    with tile.TileContext(nc) as tc, ExitStack() as ctx:
        const = ctx.enter_context(tc.tile_pool(name="const", bufs=1))
        ident_f = const.tile([P, P], F32, name="ident_f")
        nc.sync.dma_start(ident_f[:], ident_in[:])
        ones = const.tile([P, 1], F32, name="ones")
        nc.vector.memset(ones[:], 1.0)
        ones_row = const.tile([1, P], F32, name="ones_row")
        nc.vector.memset(ones_row[:], 1.0)

        _emit_half(nc, tc, bt_h, bt_l, ats_h, ats_l, b_full, a_slab, ma,
                   ones, ones_row, ident_f, "A")
        _emit_half(nc, tc, at_h, at_l, bts_h, bts_l, a_full, b_slab, mb,
                   ones, ones_row, ident_f, "B")

    split_multi_waits(nc)
    return nc


_CACHED = {}


def _get_program():
    if "nc" not in _CACHED:
        _CACHED["nc"] = build_program()
    return _CACHED["nc"]


def kernel(a_hat: np.ndarray, b_hat: np.ndarray):
    import ml_dtypes

    bf16 = ml_dtypes.bfloat16
    a_hat = np.ascontiguousarray(np.asarray(a_hat), dtype=np.float32)
    b_hat = np.ascontiguousarray(np.asarray(b_hat), dtype=np.float32)
    nc = _get_program()

    # host-side layout prep: transpose + split into exact bf16 hi/lo parts
    def split_t(x):
        xh = x.astype(bf16)
        xl = (x - xh.astype(np.float32)).astype(bf16)
        return (np.ascontiguousarray(xh.T), np.ascontiguousarray(xl.T))

    at_h, at_l = split_t(a_hat)      # [D, L] bf16
    bt_h, bt_l = split_t(b_hat)
    ident_np = np.eye(P, dtype=np.float32)

    in_maps = []
    for i in range(NB):
        sl = slice(i * S, (i + 1) * S)
        in_maps.append({
            "a_full": a_hat,
            "b_full": b_hat,
            "a_slab": np.ascontiguousarray(a_hat[sl]),
            "b_slab": np.ascontiguousarray(b_hat[sl]),
            "at_h": at_h, "at_l": at_l, "bt_h": bt_h, "bt_l": bt_l,
            "ats_h": np.ascontiguousarray(at_h[:, sl]),
            "ats_l": np.ascontiguousarray(at_l[:, sl]),
            "bts_h": np.ascontiguousarray(bt_h[:, sl]),
            "bts_l": np.ascontiguousarray(bt_l[:, sl]),
            "ident": ident_np,
        })
    res = run_bass_kernel_spmd(nc, in_maps, list(range(NB)))
    wave_a = np.concatenate([res.results[i]["ma"][0] for i in range(NB)], axis=0)
    diff_a = np.concatenate([res.results[i]["ma"][1] for i in range(NB)], axis=0)
    prod_a = np.concatenate([res.results[i]["ma"][2] for i in range(NB)], axis=0)
    wave_b = np.concatenate([res.results[i]["mb"][0] for i in range(NB)], axis=0)
    diff_b = np.concatenate([res.results[i]["mb"][1] for i in range(NB)], axis=0)
    prod_b = np.concatenate([res.results[i]["mb"][2] for i in range(NB)], axis=0)
    m_a = np.concatenate([a_hat, wave_a, diff_a, prod_a], axis=0)
    m_b = np.concatenate([b_hat, wave_b, diff_b, prod_b], axis=0)
    return (m_a, m_b)
